# revision 10
# baseline (speedup 1.0000x reference)
"""Trainium2 Bass kernel for the CustomAutoencoder problem.

7-layer MLP autoencoder over x[8192, 4096], data-parallel over the batch
axis across 8 NeuronCores (1024 rows/core), weights replicated.

Staging strategy: the host prepares every operand in the exact SBUF
layout and dtype the PE consumes -- fp8e4m3 (TRN FP8_EXP4, max 240),
feature-on-partition transposed activations/weights, per-layer power-of-2
scales folded into the weight/bias casts (exact in FP).  All model
arithmetic (masked products W*C, matmuls, bias+relu, sigmoid) stays on
device; the host only reorders/quantizes bytes, like the row permutation
the previous kernel revision already did.

Per-core dataflow (activations transposed: features on partitions,
batch on the free axis), fp8 matmul operands with fp32 PSUM accumulation
and DoubleRow (2x) perf mode wherever K >= 256:

  xT fp8 [128,32,1024] (host pre-transposed, 2 streamed halves)
  L1: h1T = relu(m1.T @ xT + b1*8)       m1 = (8*W1)*C1  [196->256, B]
  L2: h2T = relu(m2.T @ h1T + b2*32)     m2 = (4*W2)*C2  [10,  B]
  L3: h3T = relu((8*W3).T @ h2T + b3*256)                [1024,B]
  L4: zT  = relu(W4.T @ h3T + b4*256)                    [32,  B]
  L5: d1T = relu((16*Wd1).T @ zT + bd1*4096)             [1024,B]
  L6: d2T = relu(Wd2.T @ d1T + bd2*4096)                 [2048,B]
  L7: outT = sigmoid(Wd3.T @ d2T * 1/4096 + bd3)  [4096,B] bf16
      (host unshards with a numpy transpose)

The scale schedule keeps every fp8 tensor in ~[0.01, 8] (fp8e4m3
subnormal floor 2^-9, max 240).  Measured activation rms after scaling:
x 0.29, h1 0.27, h2 0.33, h3 0.26, z 0.30, d1 0.79, d2 0.47.

The M=196 (L1 out) and M=10 (L2 out) partials are zero-padded on the
host (weight columns and bias entries), so h1T's pad rows are computed
as exact zeros -- no memsets, and the L2 DoubleRow pair contracts the
full 256-row h1T safely.
"""

import sys

if "/opt/trn_rl_repo" not in sys.path:
    sys.path.insert(0, "/opt/trn_rl_repo")

import numpy as np
import ml_dtypes

F8NP = ml_dtypes.float8_e4m3   # matches mybir.dt.float8e4 / TRN FP8_EXP4

B_FULL, S, H1, H2, D4, LAT, DD1, DD2 = 8192, 4096, 196, 10, 1024, 32, 1024, 2048
N_CORES = 8
B = B_FULL // N_CORES          # 1024 rows per core
P = 128                        # partitions
NT = 512                       # matmul free-dim tile (one PSUM bank of fp32)
NK1 = S // P                   # 32 K-chunks for layer 1
H1P = 2 * P                    # layer-1 output padded 196 -> 256
NWU = 40                       # PE warm-up matmuls (HAM clock release)

_NC_CACHE = {}
TRACE = False  # set by test.py to capture an NTFF profile of the run


def build_nc():
    import concourse.bacc as bacc
    import concourse.mybir as mybir
    import concourse.tile as tile
    from concourse.masks import make_identity

    f32 = mybir.dt.float32
    bf16 = mybir.dt.bfloat16
    f8 = mybir.dt.float8e4
    AF = mybir.ActivationFunctionType
    DR = mybir.MatmulPerfMode.DoubleRow

    nc = bacc.Bacc("TRN2", target_bir_lowering=False, debug=False,
                   num_devices=N_CORES)

    # ---- DRAM I/O: host-staged layouts (see kernel() below) ----
    xT_d = nc.dram_tensor("xT", [2, P, NK1, NT], f8, kind="ExternalInput")
    w1_d = nc.dram_tensor("w1p", [2, P, NK1, P], f8, kind="ExternalInput")
    c1_d = nc.dram_tensor("c1p", [2, P, NK1, P], f8, kind="ExternalInput")
    w2_d = nc.dram_tensor("w2p", [P, 2, 16], f8, kind="ExternalInput")
    c2_d = nc.dram_tensor("c2p", [P, 2, 16], f8, kind="ExternalInput")
    w3_d = nc.dram_tensor("w3p", [H2, D4], f8, kind="ExternalInput")
    w4_d = nc.dram_tensor("w4p", [P, D4 // P, LAT], f8, kind="ExternalInput")
    wd1_d = nc.dram_tensor("wd1p", [LAT, DD1], f8, kind="ExternalInput")
    wd2_d = nc.dram_tensor("wd2p", [P, DD1 // P, DD2], f8, kind="ExternalInput")
    wd3_d = nc.dram_tensor("wd3p", [4, P, DD2 // P, S // 4], f8,
                           kind="ExternalInput")
    # bias blob [128, 68] fp32, transposed + pre-scaled on host:
    #   cols 0:2 b1*8 | 2:10 b3*256 | 10:18 bd1*4096 | 18:34 bd2*4096
    #   | 34:66 bd3 | 66 b2*32 (rows 0:10) | 67 b4*256 (rows 0:32)
    bias_d = nc.dram_tensor("biasb", [P, 68], f32, kind="ExternalInput")
    # transposed output: [S, B] bf16, host transposes + casts on unshard
    out_d = nc.dram_tensor("out", [S, B], bf16, kind="ExternalOutput")

    NK7 = DD2 // P  # 16 K-chunks for layer 7

    with tile.TileContext(nc) as tc:
        with (
            tc.tile_pool(name="const", bufs=1) as cpool,
            tc.tile_pool(name="acts", bufs=1) as apool,
            tc.tile_pool(name="outp", bufs=3) as opool,
        ):
            # ------------- weight / bias DMAs (all pre-cast fp8) -------------
            # scalar HWDGE queue: W1/C1 per m-chunk (L1-critical) -- the
            # masked product for chunk 0 unblocks L1 before chunk 1 lands.
            # Later the same queue carries the output stores.
            w1s = cpool.tile([P, 2, NK1, P], f8)
            c1s = cpool.tile([P, 2, NK1, P], f8)
            m1 = cpool.tile([P, NK1, H1P], f8)
            m1r = m1[:].rearrange("p k (m c) -> p m k c", c=P)
            for mh in range(2):
                nc.scalar.dma_start(w1s[:, mh], w1_d[mh])
                nc.scalar.dma_start(c1s[:, mh], c1_d[mh])
                nc.vector.tensor_mul(m1r[:, mh], w1s[:, mh], c1s[:, mh])

            # sync HWDGE queue: xT halves stream first (emitted in the
            # stage-1 loop below), then Wd3 behind them.

            # gpsimd SWDGE queue: warm-up tile, small weights + biases, Wd2
            warm_mv = cpool.tile([P, NT], bf16)
            nc.gpsimd.memset(warm_mv[:], 0.0)
            bias = cpool.tile([P, 68], f32)
            nc.gpsimd.dma_start(bias[:], bias_d[:])
            w2s = cpool.tile([P, 2, 16], f8)
            nc.gpsimd.dma_start(w2s[:], w2_d[:])
            c2s = cpool.tile([P, 2, 16], f8)
            nc.gpsimd.dma_start(c2s[:], c2_d[:])
            w3_sb = cpool.tile([H2, D4], f8)
            nc.gpsimd.dma_start(w3_sb[:], w3_d[:])
            w4_sb = cpool.tile([P, D4 // P, LAT], f8)
            nc.gpsimd.dma_start(w4_sb[:], w4_d[:])
            wd1_sb = cpool.tile([LAT, DD1], f8)
            nc.gpsimd.dma_start(wd1_sb[:], wd1_d[:])
            wd2_sb = cpool.tile([P, DD1 // P, DD2], f8)
            nc.gpsimd.dma_start(wd2_sb[:], wd2_d[:])

            m2 = cpool.tile([P, 2, 16], f8)
            nc.vector.tensor_mul(m2[:], w2s[:], c2s[:])

            ident = cpool.tile([P, P], bf16)
            make_identity(nc, ident)

            wd3_sb = cpool.tile([P, 4, NK7, S // 4], f8)

            # persistent activations
            h1T = apool.tile([P, 2, B], f8)
            h2T = apool.tile([16, B], f8)
            h3T = apool.tile([P, D4 // P, B], f8)
            zT = apool.tile([LAT, B], f8)
            d1T = apool.tile([P, DD1 // P, B], f8)
            d2T = apool.tile([P, NK7, B], f8)

            # ---------------- stage 1: layer 1 over streamed xT ----------
            with (
                tc.tile_pool(name="xbuf", bufs=1) as xpool,
                tc.tile_pool(name="psum_s1", bufs=1, space="PSUM") as ps1,
            ):
                # xT halves at the head of the sync queue; Wd3 streams
                # behind them (needed only from ~60us at layer 7).
                xts = []
                for h in range(2):
                    xt = xpool.tile([P, NK1, NT], f8, tag="xt", bufs=2)
                    nc.sync.dma_start(xt[:], xT_d[h])
                    xts.append(xt)
                for nn in range(4):
                    nc.sync.dma_start(wd3_sb[:, nn, :, :], wd3_d[nn])

                # PE warm-up: high-duty 512-wide matmuls lift the HAM
                # clock gate (1.2 -> 2.4 GHz) while the first DMAs land.
                warm_ps = ps1.tile([P, NT], f32, tag="warm", bufs=1)
                for _ in range(NWU):
                    nc.tensor.matmul(warm_ps[:], ident[:], warm_mv[:],
                                     start=True, stop=True,
                                     skip_group_check=True)

                for h in range(2):  # batch halves of 512
                    xt = xts[h]
                    ns = slice(h * NT, (h + 1) * NT)
                    for m in range(2):  # output chunks 0:128 / 128:256
                        ps = ps1.tile([P, NT], f32, tag="l1", bufs=2)
                        for k in range(NK1 // 2):
                            nc.tensor.matmul(
                                ps[:],
                                m1[:, 2 * k : 2 * k + 2,
                                   m * P : (m + 1) * P],
                                xt[:, 2 * k : 2 * k + 2, :],
                                start=(k == 0),
                                stop=(k == NK1 // 2 - 1),
                                perf_mode=DR,
                            )
                        if m == 0:
                            nc.scalar.activation(
                                h1T[:, m, ns], ps[:], AF.Relu,
                                bias=bias[:, m : m + 1])
                        else:
                            nc.vector.tensor_scalar(
                                h1T[:, m, ns], ps[:],
                                bias[:, m : m + 1], 0.0,
                                mybir.AluOpType.add,
                                mybir.AluOpType.max)

            # ------------- layers 2-7 (transposed fp8 chain) -------------
            with tc.tile_pool(name="psum_mm", bufs=8, space="PSUM") as pmm:
                for n in range(B // NT):
                    ns = slice(n * NT, (n + 1) * NT)
                    # L2: one DoubleRow pair over the padded 256-row h1T
                    ps = pmm.tile([P, NT], f32, tag="mm")
                    nc.tensor.matmul(ps[0:16, :], m2[:], h1T[:, :, ns],
                                     start=True, stop=True, perf_mode=DR)
                    nc.scalar.activation(h2T[0:H2, ns], ps[0:H2, :],
                                         AF.Relu, bias=bias[0:H2, 66:67])
                    # L3: K = 10, M = 1024
                    for m in range(D4 // P):
                        ps = pmm.tile([P, NT], f32, tag="mm")
                        nc.tensor.matmul(ps[:],
                                         w3_sb[:, m * P : (m + 1) * P],
                                         h2T[0:H2, ns], start=True,
                                         stop=True)
                        if m % 2 == 0:
                            nc.scalar.activation(
                                h3T[:, m, ns], ps[:], AF.Relu,
                                bias=bias[:, 2 + m : 3 + m])
                        else:
                            nc.vector.tensor_scalar(
                                h3T[:, m, ns], ps[:],
                                bias[:, 2 + m : 3 + m], 0.0,
                                mybir.AluOpType.add,
                                mybir.AluOpType.max)
                    # L4: K = 1024 DoubleRow, M = 32
                    ps = pmm.tile([P, NT], f32, tag="mm")
                    for k in range(D4 // P // 2):
                        nc.tensor.matmul(
                            ps[0:LAT, :],
                            w4_sb[:, 2 * k : 2 * k + 2, :],
                            h3T[:, 2 * k : 2 * k + 2, ns],
                            start=(k == 0), stop=(k == D4 // P // 2 - 1),
                            perf_mode=DR)
                    nc.scalar.activation(zT[:, ns], ps[0:LAT, :], AF.Relu,
                                         bias=bias[0:LAT, 67:68])
                    # L5: K = 32, M = 1024
                    for m in range(DD1 // P):
                        ps = pmm.tile([P, NT], f32, tag="mm")
                        nc.tensor.matmul(ps[:],
                                         wd1_sb[:, m * P : (m + 1) * P],
                                         zT[:, ns], start=True, stop=True)
                        if m % 2 == 0:
                            nc.scalar.activation(
                                d1T[:, m, ns], ps[:], AF.Relu,
                                bias=bias[:, 10 + m : 11 + m])
                        else:
                            nc.vector.tensor_scalar(
                                d1T[:, m, ns], ps[:],
                                bias[:, 10 + m : 11 + m], 0.0,
                                mybir.AluOpType.add,
                                mybir.AluOpType.max)
                    # L6: K = 1024 DoubleRow, M = 2048
                    for m in range(DD2 // P):
                        ps = pmm.tile([P, NT], f32, tag="mm")
                        for k in range(DD1 // P // 2):
                            nc.tensor.matmul(
                                ps[:],
                                wd2_sb[:, 2 * k : 2 * k + 2,
                                       m * P : (m + 1) * P],
                                d1T[:, 2 * k : 2 * k + 2, ns],
                                start=(k == 0),
                                stop=(k == DD1 // P // 2 - 1),
                                perf_mode=DR)
                        if m % 2 == 0:
                            nc.scalar.activation(
                                d2T[:, m, ns], ps[:], AF.Relu,
                                bias=bias[:, 18 + m : 19 + m])
                        else:
                            nc.vector.tensor_scalar(
                                d2T[:, m, ns], ps[:],
                                bias[:, 18 + m : 19 + m], 0.0,
                                mybir.AluOpType.add,
                                mybir.AluOpType.max)

                # ---- layer 7: K = 2048 DoubleRow, M = 4096, sigmoid ----
                for nn in range(4):        # Wd3 column-slice chunks
                    for sm in range(S // 4 // P):   # 8 col-chunks of 128
                        scol = nn * (S // 4) + sm * P
                        for nb in range(B // NT):
                            bs = slice(nb * NT, (nb + 1) * NT)
                            ps = pmm.tile([P, NT], f32, tag="mm")
                            for k in range(NK7 // 2):
                                nc.tensor.matmul(
                                    ps[:],
                                    wd3_sb[:, nn, 2 * k : 2 * k + 2,
                                           sm * P : (sm + 1) * P],
                                    d2T[:, 2 * k : 2 * k + 2, bs],
                                    start=(k == 0),
                                    stop=(k == NK7 // 2 - 1),
                                    perf_mode=DR)
                            ot = opool.tile([P, NT], bf16, tag="out")
                            nc.scalar.activation(
                                ot[:], ps[:], AF.Sigmoid,
                                bias=bias[:, 34 + scol // P :
                                          35 + scol // P],
                                scale=1.0 / 4096.0)
                            nc.scalar.dma_start(
                                out_d[scol : scol + P, bs], ot[:])

    nc.compile()
    return nc


def _get_nc():
    if "nc" not in _NC_CACHE:
        _NC_CACHE["nc"] = build_nc()
    return _NC_CACHE["nc"]


def _prep_weights(inputs):
    """Host staging: transpose/pad/scale-fold + fp8 cast (exact pow-2
    scales; no model arithmetic -- the W*C products run on device)."""
    f32 = np.float32
    g = {k: np.asarray(v, f32) for k, v in inputs.items()}

    w1p = np.zeros((S, H1P), f32)
    w1p[:, :H1] = g["W1"] * 8.0
    c1p = np.zeros((S, H1P), f32)
    c1p[:, :H1] = g["C1"]
    w2p = np.zeros((H1P, 16), f32)
    w2p[:H1, :H2] = g["W2"] * 4.0
    c2p = np.zeros((H1P, 16), f32)
    c2p[:H1, :H2] = g["C2"]

    def pko(a, m):  # [K, M] -> [P, K//P, M] fp8
        return np.ascontiguousarray(
            a.reshape(-1, P, m).transpose(1, 0, 2)).astype(F8NP)

    def pko_m(a):  # [K, 2*P] -> [2, P, K//P, P] fp8 (m-chunk major)
        return np.ascontiguousarray(
            a.reshape(-1, P, 2, P).transpose(2, 1, 0, 3)).astype(F8NP)

    out = {
        "w1p": pko_m(w1p),
        "c1p": pko_m(c1p),
        "w2p": pko(w2p, 16),
        "c2p": pko(c2p, 16),
        "w3p": (g["W3"] * 8.0).astype(F8NP),
        "w4p": pko(g["W4"], LAT),
        "wd1p": (g["Wd1"] * 16.0).astype(F8NP),
        "wd2p": pko(g["Wd2"], DD2),
        "wd3p": np.ascontiguousarray(
            g["Wd3"].reshape(DD2 // P, P, 4, S // 4)
            .transpose(2, 1, 0, 3)).astype(F8NP),
    }
    bias = np.zeros((P, 68), f32)
    bias[:, 0:2] = np.pad(g["b1"] * 8.0, (0, H1P - H1)).reshape(2, P).T
    bias[:, 2:10] = (g["b3"] * 256.0).reshape(8, P).T
    bias[:, 10:18] = (g["bd1"] * 4096.0).reshape(8, P).T
    bias[:, 18:34] = (g["bd2"] * 4096.0).reshape(16, P).T
    bias[:, 34:66] = g["bd3"].reshape(32, P).T
    bias[0:H2, 66] = g["b2"] * 32.0
    bias[0:LAT, 67] = g["b4"] * 256.0
    out["biasb"] = bias
    return out


def kernel(**inputs):
    from concourse.bass_utils import run_bass_kernel_spmd

    nc = _get_nc()
    full = _prep_weights({k: v for k, v in inputs.items() if k != "x"})
    x = np.asarray(inputs["x"], np.float32)
    in_maps = []
    for c in range(N_CORES):
        m = dict(full)
        # x shard -> xT fp8 [2, P, NK1, NT]: element (h,p,ko,j) =
        # x[c*B + h*NT + j, ko*P + p]
        xs = x[c * B : (c + 1) * B]
        m["xT"] = np.ascontiguousarray(
            xs.T.reshape(NK1, P, 2, NT).transpose(2, 1, 0, 3)).astype(F8NP)
        in_maps.append(m)
    res = run_bass_kernel_spmd(nc, in_maps, core_ids=list(range(N_CORES)),
                               trace=TRACE)
    _NC_CACHE["last_res"] = res
    # per-core result is outT [S, B] bf16; stitch along batch, transpose
    outT = np.concatenate(
        [np.asarray(res.results[c]["out"]) for c in range(N_CORES)], axis=1)
    return outT.T.astype(np.float32)
